# revision 25
# baseline (speedup 1.0000x reference)
"""BiAttention similarity kernel for Trainium2, 8-core data-parallel over batch.

Computes, per batch b:
    s0 = c @ c_weight                  # [L, 1]
    s1 = (c @ q_weight)^T              # [1, L]
    s2 = (c * cq_weight) @ q^T         # [L, L]
    s  = s0 + s1 + s2 + bias           # [L, L]

Shapes (hardcoded): B=8, L=2048, D=256, fp32 in/out (device math fp16/fp32,
device output fp16, upcast to fp32 on host).

Distribution: data-parallel over batch, one batch per core. Host hands each
core its shard d-major (transposed) fp16 plus pre-packed per-partition weight
tiles; device output is fp16 (halves the dominant HBM write).

Device plan per core:
  - warmup matmuls on constant data during the input-load window so the PE's
    HAM clock gate reaches 2.4 GHz before real work arrives
  - S1B [128, L]: s1[j]+bias broadcast across partitions, computed on the PE
    as qw_bcast^T @ cT (+ ones^T @ bias_row), evacuated to SBUF fp16
  - s0 column accumulated in one PSUM bank via N=1 matmuls that reuse the
    main loop's stationary cT chunks, bounced [128,1] per chunk to SBUF
  - main GEMM: 16 row chunks; PSUM tiles are [128,1024] fp32 (two banks) so
    each evacuation instruction covers two banks' worth of output
  - evacuation fuses the rank-1 adds, split across three engines:
      a tiles: DVE scalar_tensor_tensor  out = (psum + s0col) + S1B
      b tiles: ScalarE activation (psum + s0col -> fp16 tmp), then
               DVE tensor_tensor (2x fp16 mode) or GpSimd adds S1B
  - output: one 512 KiB DMA per row chunk on the Sync HWDGE ring
"""

import numpy as np
from contextlib import ExitStack

import concourse.bass as bass
import concourse.tile as tile
from concourse import bacc, mybir
from concourse.bass_utils import run_bass_kernel_spmd

F32 = mybir.dt.float32
F16 = mybir.dt.float16

B = 8
L = 2048
D = 256
NK = D // 128          # 2 contraction chunks of 128
NI = L // 128          # 16 row chunks
MAIN_N = 512           # one matmul output <= one PSUM bank
BIG_N = 1024           # evacuation tile: two PSUM banks
N_WARM = 14            # PE warmup matmuls during input load
GPS_CHUNKS = {0, 1, 2, 4, 5, 6, 8, 9, 10, 12, 13}  # b-tile partner GpSimd

TRACE = False
LAST_RESULTS = None

_NC_CACHE = None


def build_body(ctx: ExitStack, tc: tile.TileContext, aps: dict):
    nc = tc.nc
    ct_d, qt_d, w16_d, w32_d, s_d = (
        aps["ct"], aps["qt"], aps["w16"], aps["w32"], aps["s"],
    )
    Copy = mybir.ActivationFunctionType.Copy
    Ident = mybir.ActivationFunctionType.Identity
    ADD = mybir.AluOpType.add

    consts = ctx.enter_context(tc.tile_pool(name="consts", bufs=1))
    psum = ctx.enter_context(tc.tile_pool(name="psum", bufs=3, space="PSUM"))
    psum_s0 = ctx.enter_context(tc.tile_pool(name="psum_s0", bufs=1,
                                             space="PSUM"))
    outp = ctx.enter_context(tc.tile_pool(name="outp", bufs=4))
    tmpp = ctx.enter_context(tc.tile_pool(name="tmpp", bufs=4))

    # ---- packed constants: one fast HWDGE load each -----------------------
    # w16[p] = [cw[p], cw[128+p]] fp16; w32[p] = [cqw k0, cqw k1, qw k0,
    # qw k1, bias] fp32 (bias only meaningful on partition 0).
    # cT[0] is issued first on the sync ring: it gates the S1B matmuls that
    # keep the PE busy right after warmup.
    cT = [consts.tile([128, L], F16, tag=f"cT{k}", name=f"cT{k}")
          for k in range(NK)]
    w16 = consts.tile([128, NK], F16, name="w16")
    nc.sync.dma_start(w16[:], w16_d[:, :])
    w32 = consts.tile([128, 5], F32, name="w32")
    nc.sync.dma_start(w32[:], w32_d[:, :])
    nc.sync.dma_start(cT[0][:], ct_d[0:128, :])
    nc.sync.dma_start(cT[1][:], ct_d[128:256, :])
    cw16 = w16
    cqw32 = w32[:, 0:NK]
    qw32 = w32[:, NK:2 * NK]
    bias_sb = w32[0:1, 4:5]

    ones_sb = consts.tile([128, MAIN_N], F16, name="ones_sb")
    nc.vector.memset(ones_sb[:], 1.0)

    # ---- PE warmup: release the HAM clock gate during the load window ----
    # The s0acc banks double as the warmup target; junk lands in columns the
    # s0 matmuls never touch (and start=True clears has_written anyway).
    # Two s0acc banks ping-pong across chunks so the PE's s0 matmul of chunk
    # i only serializes against the DVE bounce of chunk i-2 (2 chunks slack).
    s0acc = [psum_s0.tile([128, MAIN_N], F32, tag=f"s0acc{t}",
                          name=f"s0acc{t}") for t in range(2)]
    for w in range(N_WARM):
        nc.tensor.matmul(s0acc[w % 2][:], ones_sb[:, 0:128], ones_sb[:],
                         start=True, stop=True)

    # ---- transposed fp16 operands ----------------------------------------
    qT = [consts.tile([128, L], F16, tag=f"qT{k}", name=f"qT{k}")
          for k in range(NK)]
    # qw_bc / bias_row first in the DVE/ACT FIFOs: they only need w32+ones,
    # so S1B's k=0 matmuls can start as soon as cT[0] lands.
    qw_bc = [consts.tile([128, 128], F16, tag=f"qwbc{k}", name=f"qwbc{k}")
             for k in range(NK)]
    for k in range(NK):
        nc.vector.tensor_scalar_mul(qw_bc[k][:], ones_sb[:, 0:128],
                                    qw32[:, k:k + 1])
    bias_row = consts.tile([1, MAIN_N], F16, name="bias_row")
    nc.scalar.activation(bias_row[0:1, :], ones_sb[0:1, :], Copy,
                         scale=bias_sb)

    for k in range(NK):
        ksl = slice(k * 128, (k + 1) * 128)
        nc.scalar.dma_start(qT[k][:], qt_d[ksl, :])
        # qmodT = qT * cq_weight (per-partition scalar after transpose)
        nc.vector.tensor_scalar_mul(qT[k][:], qT[k][:], cqw32[:, k:k + 1])

    # ---- S1B: s1[j] + bias broadcast across all 128 partitions -----------
    # Accumulation order k0 -> bias -> k1 keeps the PE busy on work that only
    # needs cT[0] while cT[1] is still in flight.
    s1b = consts.tile([128, L], F16, name="s1b")
    s1b_ps = [psum.tile([128, BIG_N], F32, tag="main", name=f"s1b_ps{t}")
              for t in range(2)]
    for jj in range(4):
        jsl = slice((jj % 2) * MAIN_N, (jj % 2 + 1) * MAIN_N)
        nc.tensor.matmul(s1b_ps[jj // 2][:, jsl], qw_bc[0][:],
                         cT[0][:, jj * MAIN_N:(jj + 1) * MAIN_N],
                         start=True, stop=False)
    for jj in range(4):
        jsl = slice((jj % 2) * MAIN_N, (jj % 2 + 1) * MAIN_N)
        nc.tensor.matmul(s1b_ps[jj // 2][:, jsl], ones_sb[0:1, 0:128],
                         bias_row[0:1, :], start=False, stop=False)
    for jj in range(4):
        jsl = slice((jj % 2) * MAIN_N, (jj % 2 + 1) * MAIN_N)
        nc.tensor.matmul(s1b_ps[jj // 2][:, jsl], qw_bc[1][:],
                         cT[1][:, jj * MAIN_N:(jj + 1) * MAIN_N],
                         start=False, stop=True)
    nc.vector.tensor_copy(s1b[:, 0:BIG_N], s1b_ps[0][:])
    nc.scalar.activation(s1b[:, BIG_N:L], s1b_ps[1][:], Copy)

    s0col = consts.tile([128, NI], F32, name="s0col")

    # ---- main loop: 16 row chunks ----------------------------------------
    for i in range(NI):
        isl = slice(i * 128, (i + 1) * 128)
        out_sb = outp.tile([128, L], F16, tag="out", name="out_sb")
        pa = psum.tile([128, BIG_N], F32, tag="main", name="pa")
        pb = psum.tile([128, BIG_N], F32, tag="main", name="pb")
        halves = [pa[:, 0:MAIN_N], pa[:, MAIN_N:BIG_N],
                  pb[:, 0:MAIN_N], pb[:, MAIN_N:BIG_N]]
        # s0's N=1 matmul leads each k-group so the s0 column completes four
        # main passes before chunk end, giving the bounce -> stt -> TT chain
        # a head start over the next chunk's matmuls.
        for k in range(NK):
            nc.tensor.matmul(s0acc[i % 2][:, i // 2:i // 2 + 1],
                             cT[k][:, isl], cw16[:, k:k + 1],
                             start=(k == 0), stop=(k == NK - 1))
            for jj in range(4):
                nc.tensor.matmul(halves[jj], cT[k][:, isl],
                                 qT[k][:, jj * MAIN_N:(jj + 1) * MAIN_N],
                                 start=(k == 0), stop=(k == NK - 1))
        # bounce s0 column through SBUF for the evacuation ops (on ScalarE,
        # keeping the DVE free to start the fused evacuation promptly)
        nc.scalar.activation(s0col[:, i:i + 1],
                             s0acc[i % 2][:, i // 2:i // 2 + 1], Copy)
        # a-tile: fused three-term evacuation on DVE
        nc.vector.scalar_tensor_tensor(out_sb[:, 0:BIG_N], pa[:],
                                       s0col[:, i:i + 1],
                                       s1b[:, 0:BIG_N], ADD, ADD)
        # b-tile: ScalarE folds s0 in (bias) while converting to fp16, then
        # the partner engine adds S1B with a plain tensor_tensor, which has
        # a 2x fp16 uop (scalar_tensor_tensor does not).
        tmp = tmpp.tile([128, BIG_N], F16, tag="tmp", name="tmp")
        nc.scalar.activation(tmp[:], pb[:], Ident, bias=s0col[:, i:i + 1])
        if i in GPS_CHUNKS:
            nc.gpsimd.tensor_tensor(out_sb[:, BIG_N:L], tmp[:],
                                    s1b[:, BIG_N:L], ADD)
        else:
            nc.vector.tensor_tensor(out_sb[:, BIG_N:L], tmp[:],
                                    s1b[:, BIG_N:L], ADD)
        nc.sync.dma_start(s_d[isl, :], out_sb[:])


def build_nc():
    nc = bacc.Bacc("TRN2", target_bir_lowering=False, debug=False)
    aps = {
        "ct": nc.dram_tensor("ct", [D, L], F16, kind="ExternalInput").ap(),
        "qt": nc.dram_tensor("qt", [D, L], F16, kind="ExternalInput").ap(),
        "w16": nc.dram_tensor("w16", [128, NK], F16, kind="ExternalInput").ap(),
        "w32": nc.dram_tensor("w32", [128, 5], F32, kind="ExternalInput").ap(),
        "s": nc.dram_tensor("s", [L, L], F16, kind="ExternalOutput").ap(),
    }
    with tile.TileContext(nc) as tc:
        with ExitStack() as ctx:
            build_body(ctx, tc, aps)
    nc.compile()
    return nc


def get_nc():
    global _NC_CACHE
    if _NC_CACHE is None:
        _NC_CACHE = build_nc()
    return _NC_CACHE


def kernel(c, q, c_weight, q_weight, cq_weight, bias):
    global LAST_RESULTS
    nc = get_nc()
    c = np.asarray(c, dtype=np.float32)
    q = np.asarray(q, dtype=np.float32)
    cw = np.asarray(c_weight, dtype=np.float32).reshape(D)
    qw = np.asarray(q_weight, dtype=np.float32).reshape(D)
    cqw = np.asarray(cq_weight, dtype=np.float32).reshape(D)
    bias = np.asarray(bias, dtype=np.float32).reshape(1)

    # packed per-partition weights: row p of w16 = [cw[p], cw[128+p]] fp16;
    # row p of w32 = [cqw[p], cqw[128+p], qw[p], qw[128+p], bias]
    w16 = np.ascontiguousarray(cw.reshape(NK, 128).T).astype(np.float16)
    w32 = np.empty((128, 5), dtype=np.float32)
    w32[:, 0:NK] = cqw.reshape(NK, 128).T
    w32[:, NK:2 * NK] = qw.reshape(NK, 128).T
    w32[:, 4] = bias[0]

    in_maps = [
        {
            "ct": np.ascontiguousarray(c[b].T).astype(np.float16),
            "qt": np.ascontiguousarray(q[b].T).astype(np.float16),
            "w16": w16,
            "w32": w32,
        }
        for b in range(B)
    ]
    res = run_bass_kernel_spmd(nc, in_maps, core_ids=list(range(B)), trace=TRACE)
    LAST_RESULTS = res
    return np.stack([res.results[b]["s"].astype(np.float32) for b in range(B)],
                    axis=0)
